# revision 14
# baseline (speedup 1.0000x reference)
"""Trainium2 Bass kernel for nn_Net_420906795534 (GNN: 3x GraphConv + TopKPooling + readout + MLP).

Sharding: data-parallel over graphs - 8 graphs per NeuronCore x 8 cores.
Host does index-only preprocessing: per-graph dense adjacency count matrices
(bf16, exact since max multiplicity is 3) and layout reshapes. All float
compute (convs, pooling, readouts, MLP) runs on device.

Device algorithm (per graph, nodes stay in fixed slots, no compaction):
  conv:    agg_T[f,d] = sum_c h_nm[c](f32r).T @ A[c](bf16)   (PE, A streamed)
           h_T = relu(W_rel.T @ agg_T + W_root.T @ h'_T + b)  (PE + ACT)
  pool:    u = (h.w)/||w|| ; selection replicates jax.lax.top_k EXACTLY:
           scores tie at +-1 (fp32 tanh saturation, |u| >= 7.99881172...),
           ties break by previous-layer compaction order = lexicographic
           (u_l desc, u_{l-1} desc, ..., u_1 desc, node-index asc).
           Implemented as a cascade of exact rank-R extractions via the
           gpsimd kth_largest instruction at a static rank R = n_drop.
  readout: masked max (strided reduce + PE transpose), sum via ones-column
           matmuls; mean = sum/k with static k. z = x1+x2+x3 -> 3-layer MLP.
"""
import sys
sys.path.insert(0, '/opt/trn_rl_repo')
import math
import numpy as np
import ml_dtypes

B_GRAPHS, N, DEG = 64, 1024, 16
IN_F, HID = 20, 128
G_PER_CORE = 8
N_CORES = 8
P = 128
NCH = N // P  # 8 node chunks per graph
XSAT = np.float32(7.998811721801758)  # XLA-cpu f32 tanh saturation cutoff
K1, K2, K3 = 820, 656, 525           # ceil(0.8*n) chain
NDROP = {1: N - K1, 2: K1 - K2, 3: K2 - K3}      # 204, 164, 131
NVALID = {1: N, 2: K1, 3: K2}
KKEEP = {1: K1, 2: K2, 3: K3}


def _quantile_for_rank(rank_m2: int, n_valid: int) -> float:
    """Return q so kth_largest's k_adj == rank_m2 exactly (frac irrelevant:
    we read out[1] = desc[k_adj+1])."""
    lo = int(math.ceil(rank_m2 * (1 << 32) / (n_valid - 1)))
    hi = int(math.ceil((rank_m2 + 1) * (1 << 32) / (n_valid - 1))) - 1
    omq = (lo + hi) // 2
    assert (omq * (n_valid - 1)) >> 32 == rank_m2
    return 1.0 - omq / (1 << 32)


def build_program():
    import concourse.bacc as bacc
    import concourse.mybir as mybir
    import concourse.tile as tile
    from concourse.masks import make_identity

    f32 = mybir.dt.float32
    bf16 = mybir.dt.bfloat16
    i32 = mybir.dt.int32
    AF = mybir.ActivationFunctionType
    ALU = mybir.AluOpType
    AX = mybir.AxisListType

    nc = bacc.Bacc("TRN2", target_bir_lowering=False, debug=False,
                   num_devices=N_CORES)

    # ---------------- DRAM I/O ----------------
    d_x = nc.dram_tensor("x_nm", [G_PER_CORE, P, NCH * IN_F], f32, kind="ExternalInput")
    d_A = nc.dram_tensor("A_sd", [G_PER_CORE, P, NCH * N], bf16, kind="ExternalInput")
    d_w = {}
    for l, infl in ((1, IN_F), (2, HID), (3, HID)):
        d_w[f"W_rel{l}"] = nc.dram_tensor(f"W_rel{l}", [infl, HID], f32, kind="ExternalInput")
        d_w[f"W_root{l}"] = nc.dram_tensor(f"W_root{l}", [infl, HID], f32, kind="ExternalInput")
        d_w[f"b_rel{l}"] = nc.dram_tensor(f"b_rel{l}", [HID, 1], f32, kind="ExternalInput")
        d_w[f"w_pool{l}"] = nc.dram_tensor(f"w_pool{l}", [HID, 1], f32, kind="ExternalInput")
    d_w["W_rel1s"] = nc.dram_tensor("W_rel1s", [2 * IN_F, HID], f32, kind="ExternalInput")
    d_w["W_lin1a"] = nc.dram_tensor("W_lin1a", [HID, HID], f32, kind="ExternalInput")
    d_w["W_lin1b"] = nc.dram_tensor("W_lin1b", [HID, HID], f32, kind="ExternalInput")
    d_w["b_lin1"] = nc.dram_tensor("b_lin1", [HID, 1], f32, kind="ExternalInput")
    d_w["W_lin2"] = nc.dram_tensor("W_lin2", [HID, 64], f32, kind="ExternalInput")
    d_w["b_lin2"] = nc.dram_tensor("b_lin2", [64, 1], f32, kind="ExternalInput")
    d_w["W_lin3"] = nc.dram_tensor("W_lin3", [64, 1], f32, kind="ExternalInput")
    d_w["b_lin3"] = nc.dram_tensor("b_lin3", [1, 1], f32, kind="ExternalInput")
    d_out = nc.dram_tensor("out", [1, G_PER_CORE], f32, kind="ExternalOutput")

    with tile.TileContext(nc) as tc:
        with (
            tc.tile_pool(name="const", bufs=1) as cpool,
            tc.tile_pool(name="apool", bufs=2) as apool,
            tc.tile_pool(name="hpool", bufs=4) as hpool,
            tc.tile_pool(name="small", bufs=4) as spool,
            tc.tile_pool(name="tiny", bufs=6) as tpool,
            tc.tile_pool(name="psA", bufs=2, space="PSUM") as psA,
            tc.tile_pool(name="psT", bufs=2, space="PSUM") as psT,
            tc.tile_pool(name="psS", bufs=2, space="PSUM") as psS,
        ):
            # ---------- constants / weights ----------
            ident = cpool.tile([P, P], f32)
            make_identity(nc, ident[:])
            ones_bf = cpool.tile([P, 1], bf16)
            nc.vector.memset(ones_bf[:], 1.0)
            idxb = cpool.tile([P, NCH], f32)
            idxb_i = cpool.tile([P, NCH], i32)
            nc.gpsimd.iota(idxb_i[:], pattern=[[128, NCH]], base=0, channel_multiplier=1)
            nc.vector.tensor_copy(idxb[:], idxb_i[:])

            w_t = {}
            for name, dd in d_w.items():
                t = cpool.tile(list(dd.shape), f32, tag=name)
                nc.sync.dma_start(out=t[:], in_=dd[:])
                w_t[name] = t

            # invnorm_l = 1/||w_pool_l|| replicated [P,1]
            invnorm = {}
            for l in (1, 2, 3):
                pnw = psS.tile([1, 1], f32, tag="s")
                nc.tensor.matmul(pnw[:], lhsT=w_t[f"w_pool{l}"][:], rhs=w_t[f"w_pool{l}"][:],
                                 start=True, stop=True)
                nrm = tpool.tile([1, 1], f32, tag="nrm")
                nc.scalar.activation(nrm[:], pnw[:], AF.Sqrt)
                inv = tpool.tile([1, 1], f32, tag="inv")
                nc.vector.reciprocal(inv[:], nrm[:])
                invr = cpool.tile([P, 1], f32, tag=f"invn{l}")
                nc.gpsimd.partition_broadcast(invr[:], inv[:], channels=P)
                invnorm[l] = invr

            # global readout accumulators [feat, graph]
            zmax = cpool.tile([P, G_PER_CORE], f32)
            zmean = cpool.tile([P, G_PER_CORE], f32)
            nc.vector.memset(zmax[:], 0.0)
            nc.vector.memset(zmean[:], 0.0)

            BIG = 1e20
            INVALID = -1e30

            def graph_chain(g):
                # ---------- load graph ----------
                t_x = spool.tile([P, NCH * IN_F], f32, tag="x")
                nc.sync.dma_start(out=t_x[:], in_=d_x[g])
                xs = spool.tile([P, NCH * 2 * IN_F], bf16, tag="xs")
                xs3 = xs[:].rearrange("p (c t) -> p c t", t=2 * IN_F)
                x_hi = xs3[:, :, 0:IN_F]
                x_lo = xs3[:, :, IN_F:2 * IN_F]
                nc.vector.tensor_copy(x_hi, t_x[:].rearrange("p (c t) -> p c t", t=IN_F))
                x_hif = spool.tile([P, NCH * IN_F], f32, tag="xhif")
                nc.vector.tensor_copy(x_hif[:], x_hi)
                x_lo_f = spool.tile([P, NCH * IN_F], f32, tag="xlof")
                nc.vector.tensor_tensor(out=x_lo_f[:], in0=t_x[:], in1=x_hif[:], op=ALU.subtract)
                nc.vector.tensor_copy(x_lo, x_lo_f[:].rearrange("p (c t) -> p c t", t=IN_F))
                t_A = apool.tile([P, NCH * N], bf16, tag="A")
                nc.sync.dma_start(out=t_A[:], in_=d_A[g])

                # x_T [IN_F, N] via PE transpose of the 8 chunks
                pxT = psA.tile([IN_F, N], f32, tag="agg")
                for c in range(NCH):
                    nc.tensor.transpose(
                        pxT[:, c * P:(c + 1) * P],
                        t_x[:, c * IN_F:(c + 1) * IN_F],
                        ident[:],
                    )
                xT = spool.tile([IN_F, N], f32, tag="xT")
                nc.scalar.copy(xT[:], pxT[:])

                keep = tpool.tile([P, NCH], f32, tag="keep")
                nc.vector.memset(keep[:], 1.0)
                ucs = []
                h_hi = h_lo = None        # layer 1 uses stacked xs
                hT = xT                   # f32 feature-major [infl, N]
                infl = IN_F

                for l in (1, 2, 3):
                    nvalid, ndrop, kkeep = NVALID[l], NDROP[l], KKEEP[l]
                    # ---------- conv: agg_T = sum_c (hi_c + lo_c).T @ A_c ----------
                    if l == 1:
                        # stacked [x_hi | x_lo] lhsT: one A pass, psum rows
                        # 0:IN_F = hi part, IN_F:2*IN_F = lo part (summed by
                        # the stacked-W_rel1 linear contraction below)
                        agg_rows = 2 * IN_F
                        parts = [(xs, agg_rows)]
                    else:
                        agg_rows = infl
                        parts = [(h_hi, infl), (h_lo, infl)]
                    pagg = psA.tile([agg_rows, N], f32, tag="agg")
                    npart = len(parts)
                    for half in range(2):
                        rsl = slice(half * 512, (half + 1) * 512)
                        for c in range(NCH):
                            for hl, (part, pw) in enumerate(parts):
                                nc.tensor.matmul(
                                    pagg[:, rsl],
                                    lhsT=part[:, c * pw:(c + 1) * pw],
                                    rhs=t_A[:, c * N + half * 512: c * N + (half + 1) * 512],
                                    start=(c == 0 and hl == 0), stop=(c == NCH - 1 and hl == npart - 1),
                                    skip_group_check=True)
                    if l >= 2:
                        psum_prev = psS.tile([infl, 1], f32, tag="s")
                        for c in range(NCH):
                            for hl, part in ((0, h_hi), (1, h_lo)):
                                nc.tensor.matmul(
                                    psum_prev[:], lhsT=part[:, c * infl:(c + 1) * infl],
                                    rhs=ones_bf[:], start=(c == 0 and hl == 0),
                                    stop=(c == NCH - 1 and hl == 1),
                                    skip_group_check=True)
                        nc.vector.scalar_tensor_tensor(
                            out=zmean[:, g:g + 1], in0=psum_prev[:],
                            scalar=1.0 / KKEEP[l - 1], in1=zmean[:, g:g + 1],
                            op0=ALU.mult, op1=ALU.add)
                    aggT = spool.tile([agg_rows, N], f32, tag="aggT")
                    nc.scalar.copy(aggT[:], pagg[:])

                    # ---------- linear (pure fp32): h_T = relu(Wrel.T@aggT + Wroot.T@hT + b) ----------
                    ph = psA.tile([HID, N], f32, tag="agg")
                    for half in range(2):
                        sl = slice(half * 512, (half + 1) * 512)
                        relw = "W_rel1s" if l == 1 else f"W_rel{l}"
                        nc.tensor.matmul(ph[:, sl], lhsT=w_t[relw][:],
                                         rhs=aggT[:, sl],
                                         start=True, stop=False, skip_group_check=True)
                        nc.tensor.matmul(ph[:, sl], lhsT=w_t[f"W_root{l}"][:],
                                         rhs=hT[:, sl],
                                         start=False, stop=True, skip_group_check=True)
                    hT_new = hpool.tile([HID, N], f32, tag="hT")
                    nc.scalar.activation(hT_new[:], ph[:], AF.Relu, bias=w_t[f"b_rel{l}"][:, 0:1])

                    # ---------- scores ----------
                    pz = psS.tile([P, NCH], f32, tag="s")
                    for c in range(NCH):
                        nc.tensor.matmul(
                            pz[:, c:c + 1],
                            lhsT=hT_new[:, c * P:(c + 1) * P],
                            rhs=w_t[f"w_pool{l}"][:],
                            start=(c == 0), stop=(c == NCH - 1), skip_group_check=True)
                    u = tpool.tile([P, NCH], f32, tag="u")
                    nc.scalar.activation(u[:], pz[:], AF.Copy, scale=invnorm[l][:, 0:1])
                    uc = tpool.tile([P, NCH], f32, tag=f"uc{l}_{g % 2}")
                    nc.vector.tensor_scalar(out=uc[:], in0=u[:], scalar1=float(XSAT),
                                            scalar2=float(-XSAT), op0=ALU.min, op1=ALU.max)
                    ucs.append(uc)

                    # ---------- exact top-k keep mask (lex cascade) ----------
                    comps = [("u", t) for t in reversed(ucs)] + [("i", idxb)]
                    bg = tpool.tile([P, NCH], f32, tag="bg")
                    nc.vector.tensor_scalar(out=bg[:], in0=keep[:], scalar1=float(-INVALID),
                                            scalar2=float(INVALID), op0=ALU.mult, op1=ALU.add)
                    ic = tpool.tile([P, NCH], f32, tag="ic")
                    nc.vector.tensor_copy(ic[:], keep[:])
                    dropped = tpool.tile([P, NCH], f32, tag="dropped")
                    nc.vector.memset(dropped[:], 0.0)
                    q = _quantile_for_rank(ndrop - 2, nvalid)
                    for j, (kind, comp) in enumerate(comps):
                        key = tpool.tile([P, NCH], f32, tag="key")
                        nc.vector.tensor_tensor(out=key[:], in0=comp[:], in1=ic[:], op=ALU.mult)
                        if kind == "u":
                            nc.vector.scalar_tensor_tensor(out=key[:], in0=key[:], scalar=-1.0,
                                                           in1=bg[:], op0=ALU.mult, op1=ALU.add)
                        else:
                            nc.vector.tensor_tensor(out=key[:], in0=key[:], in1=bg[:], op=ALU.add)
                        tv = tpool.tile([1, 2], f32, tag="tv")
                        nc.gpsimd.kth_largest(tv[:], key[:], n_per_lane=NCH, k=ndrop,
                                              quantile=q)
                        vrep = tpool.tile([P, 1], f32, tag="vrep")
                        nc.gpsimd.partition_broadcast(vrep[:], tv[:, 1:2], channels=P)
                        last = (j == len(comps) - 1)
                        nd = tpool.tile([P, NCH], f32, tag="nd")
                        nc.vector.tensor_tensor(
                            out=nd[:], in0=key[:],
                            in1=vrep[:, 0:1].to_broadcast([P, NCH]),
                            op=(ALU.is_ge if last else ALU.is_gt))
                        nc.vector.tensor_tensor(out=nd[:], in0=nd[:], in1=ic[:], op=ALU.mult)
                        nc.vector.tensor_tensor(out=dropped[:], in0=dropped[:], in1=nd[:], op=ALU.add)
                        if not last:
                            eq = tpool.tile([P, NCH], f32, tag="eq")
                            nc.vector.tensor_tensor(
                                out=eq[:], in0=key[:],
                                in1=vrep[:, 0:1].to_broadcast([P, NCH]), op=ALU.is_equal)
                            ic_new = tpool.tile([P, NCH], f32, tag="ic")
                            nc.vector.tensor_tensor(out=ic_new[:], in0=eq[:], in1=ic[:], op=ALU.mult)
                            safe = tpool.tile([P, NCH], f32, tag="safe")
                            nc.vector.tensor_tensor(out=safe[:], in0=ic[:], in1=ic_new[:], op=ALU.subtract)
                            nc.vector.tensor_tensor(out=safe[:], in0=safe[:], in1=nd[:], op=ALU.subtract)
                            nc.vector.scalar_tensor_tensor(out=bg[:], in0=nd[:], scalar=float(BIG),
                                                           in1=bg[:], op0=ALU.mult, op1=ALU.add)
                            nc.vector.scalar_tensor_tensor(out=bg[:], in0=safe[:], scalar=float(-BIG),
                                                           in1=bg[:], op0=ALU.mult, op1=ALU.add)
                            ic = ic_new
                    keep_new = tpool.tile([P, NCH], f32, tag="keep")
                    nc.vector.tensor_tensor(out=keep_new[:], in0=keep[:], in1=dropped[:], op=ALU.subtract)
                    keep = keep_new

                    # ---------- scale + masked variants ----------
                    s = tpool.tile([P, NCH], f32, tag="s")
                    nc.scalar.activation(s[:], u[:], AF.Tanh)
                    sk = tpool.tile([P, NCH], f32, tag="sk")
                    nc.vector.tensor_tensor(out=sk[:], in0=s[:], in1=keep[:], op=ALU.mult)
                    maskadd = tpool.tile([P, NCH], f32, tag="maskadd")
                    nc.vector.tensor_scalar(out=maskadd[:], in0=keep[:], scalar1=float(-INVALID),
                                            scalar2=float(INVALID), op0=ALU.mult, op1=ALU.add)

                    # transpose h_T -> node-major; h'_f32, hi/lo bf16, masked f32
                    hp32 = hpool.tile([P, NCH * HID], f32, tag="hp32")
                    h_hi_n = hpool.tile([P, NCH * HID], bf16, tag="hhi")
                    h_lo_n = hpool.tile([P, NCH * HID], bf16, tag="hlo")
                    hm_nm = hpool.tile([P, NCH * HID], f32, tag="hmnm")
                    for c in range(NCH):
                        csl = slice(c * HID, (c + 1) * HID)
                        pt = psT.tile([P, P], f32, tag="pt")
                        nc.tensor.transpose(pt[:], hT_new[:, c * P:(c + 1) * P], ident[:])
                        nc.scalar.activation(hp32[:, csl], pt[:], AF.Copy, scale=sk[:, c:c + 1])
                        nc.gpsimd.tensor_copy(h_hi_n[:, csl], hp32[:, csl])
                        nc.vector.tensor_tensor(out=h_lo_n[:, csl], in0=hp32[:, csl],
                                                in1=h_hi_n[:, csl], op=ALU.subtract)
                        nc.gpsimd.tensor_tensor(
                            out=hm_nm[:, csl], in0=hp32[:, csl],
                            in1=maskadd[:, c:c + 1].to_broadcast([P, HID]),
                            op=ALU.add)

                    # ---------- max readout ----------
                    pmax = tpool.tile([P, HID], f32, tag="pmax")
                    nc.vector.tensor_reduce(
                        out=pmax[:], in_=hm_nm[:].rearrange("p (c f) -> p f c", c=NCH),
                        axis=AX.X, op=ALU.max)
                    ptm = psT.tile([P, P], f32, tag="pt")
                    nc.tensor.transpose(ptm[:], pmax[:], ident[:])
                    gmax = tpool.tile([P, 1], f32, tag="gmax")
                    nc.vector.tensor_reduce(out=gmax[:], in_=ptm[:], axis=AX.X, op=ALU.max)
                    nc.vector.tensor_tensor(out=zmax[:, g:g + 1], in0=zmax[:, g:g + 1],
                                            in1=gmax[:], op=ALU.add)

                    # h'_T for next layer's root term
                    if l < 3:
                        hpT = hpool.tile([HID, N], f32, tag="hpT")
                        for c in range(NCH):
                            pt2 = psT.tile([P, P], f32, tag="pt")
                            nc.tensor.transpose(pt2[:], hp32[:, c * HID:(c + 1) * HID], ident[:])
                            nc.scalar.copy(hpT[:, c * P:(c + 1) * P], pt2[:])
                        hT = hpT
                    h_hi, h_lo = h_hi_n, h_lo_n
                    infl = HID
                    yield

                # layer-3 sum readout
                ps3 = psS.tile([HID, 1], f32, tag="s")
                for c in range(NCH):
                    for hl, part in ((0, h_hi), (1, h_lo)):
                        nc.tensor.matmul(ps3[:], lhsT=part[:, c * HID:(c + 1) * HID],
                                         rhs=ones_bf[:], start=(c == 0 and hl == 0),
                                         stop=(c == NCH - 1 and hl == 1),
                                         skip_group_check=True)
                nc.vector.scalar_tensor_tensor(out=zmean[:, g:g + 1], in0=ps3[:],
                                               scalar=1.0 / K3, in1=zmean[:, g:g + 1],
                                               op0=ALU.mult, op1=ALU.add)
                yield

            # software-pipeline graphs in pairs: interleave the two chains'
            # layer stages in emission order so one graph's PE work fills the
            # other's ACT/selection-cascade gaps
            for base in range(0, G_PER_CORE, 2):
                ga = graph_chain(base)
                gb = graph_chain(base + 1)
                for _ in range(4):
                    next(ga, None)
                    next(gb, None)

            # ---------------- MLP over all graphs (fp32) ----------------
            pa1 = psS.tile([HID, G_PER_CORE], f32, tag="s")
            nc.tensor.matmul(pa1[:], lhsT=w_t["W_lin1a"][:],
                             rhs=zmax[:], start=True, stop=False,
                             skip_group_check=True)
            nc.tensor.matmul(pa1[:], lhsT=w_t["W_lin1b"][:],
                             rhs=zmean[:], start=False, stop=True,
                             skip_group_check=True)
            a1 = spool.tile([HID, G_PER_CORE], f32, tag="a1")
            nc.scalar.activation(a1[:], pa1[:], AF.Relu, bias=w_t["b_lin1"][:, 0:1])
            pa2 = psS.tile([64, G_PER_CORE], f32, tag="s")
            nc.tensor.matmul(pa2[:], lhsT=w_t["W_lin2"][:],
                             rhs=a1[:], start=True, stop=True)
            a2 = spool.tile([64, G_PER_CORE], f32, tag="a2")
            nc.scalar.activation(a2[:], pa2[:], AF.Relu, bias=w_t["b_lin2"][:, 0:1])
            pa3 = psS.tile([1, G_PER_CORE], f32, tag="s")
            nc.tensor.matmul(pa3[:], lhsT=w_t["W_lin3"][:],
                             rhs=a2[:], start=True, stop=True)
            a3 = spool.tile([1, G_PER_CORE], f32, tag="a3")
            nc.scalar.activation(a3[:], pa3[:], AF.Identity, bias=w_t["b_lin3"][:, 0:1])
            nc.sync.dma_start(out=d_out[:], in_=a3[:])

    nc.compile()
    return nc


def prepare_inputs(inputs):
    """Host index-preprocessing + sharding. Returns per-core input maps."""
    x = np.asarray(inputs["x"], np.float32)
    ei = np.asarray(inputs["edge_index"], np.int64)
    src = ei[0] % N
    dst = ei[1] % N
    gid = ei[0] // N

    maps = []
    for core in range(N_CORES):
        gs = range(core * G_PER_CORE, (core + 1) * G_PER_CORE)
        xs = np.empty((G_PER_CORE, P, NCH * IN_F), np.float32)
        As = np.empty((G_PER_CORE, P, NCH * N), ml_dtypes.bfloat16)
        for i, g in enumerate(gs):
            xg = x[g * N:(g + 1) * N].reshape(NCH, P, IN_F).transpose(1, 0, 2)
            xs[i] = xg.reshape(P, NCH * IN_F)
            m = gid == g
            A = np.zeros((N, N), np.float32)
            np.add.at(A, (src[m], dst[m]), 1.0)
            As[i] = (A.reshape(NCH, P, N).transpose(1, 0, 2)
                      .reshape(P, NCH * N).astype(ml_dtypes.bfloat16))
        im = {"x_nm": xs, "A_sd": As}
        for l in (1, 2, 3):
            im[f"W_rel{l}"] = np.asarray(inputs[f"W_rel{l}"], np.float32)
            im[f"W_root{l}"] = np.asarray(inputs[f"W_root{l}"], np.float32)
            im[f"b_rel{l}"] = np.asarray(inputs[f"b_rel{l}"], np.float32).reshape(HID, 1)
            im[f"w_pool{l}"] = np.asarray(inputs[f"w_pool{l}"], np.float32).reshape(HID, 1)
        Wr1 = np.asarray(inputs["W_rel1"], np.float32)
        im["W_rel1s"] = np.vstack([Wr1, Wr1])
        W1 = np.asarray(inputs["W_lin1"], np.float32)
        im["W_lin1a"] = np.ascontiguousarray(W1[:HID])
        im["W_lin1b"] = np.ascontiguousarray(W1[HID:])
        im["b_lin1"] = np.asarray(inputs["b_lin1"], np.float32).reshape(HID, 1)
        im["W_lin2"] = np.asarray(inputs["W_lin2"], np.float32)
        im["b_lin2"] = np.asarray(inputs["b_lin2"], np.float32).reshape(64, 1)
        im["W_lin3"] = np.asarray(inputs["W_lin3"], np.float32)
        im["b_lin3"] = np.asarray(inputs["b_lin3"], np.float32).reshape(1, 1)
        maps.append(im)
    return maps


_RESULTS_CACHE = {}


def run_on_device(inputs, trace=False):
    from concourse.bass_utils import run_bass_kernel_spmd
    nc = build_program()
    maps = prepare_inputs(inputs)
    res = run_bass_kernel_spmd(nc, maps, core_ids=list(range(N_CORES)),
                               trace=trace)
    outs = [res.results[c]["out"].reshape(-1) for c in range(N_CORES)]
    full = np.concatenate(outs).astype(np.float32).reshape(B_GRAPHS, 1)
    return full, res


def kernel(**inputs) -> np.ndarray:
    out, _ = run_on_device(inputs)
    return out

